# revision 29
# baseline (speedup 1.0000x reference)
"""Trainium2 Bass kernel for differentiable KDE (Gaussian kernel density estimate).

Math (h = 1):
    sq[i,j]    = ||x_i||^2 + ||d_j||^2 - 2 x_i.d_j
    density[i] = mean_j exp(-C * sq[i,j]),   C = 0.5 / sqrt(2*pi)
               = exp(-C||x_i||^2 - lnM - (SW-SE))
                 * sum_j [exp(-C||d_j||^2 + SW) * exp(2C x_i.d_j - SE)]
    with constant range-shifts SE, SW so both exp factors fit bf16 comfortably.

Sharding: data-parallel over x rows (1024 per core), data replicated.

Per-core pipeline (j = data row as PSUM partition, i = x row as free dim):
    - Contiguous DMA chunks (4KB/partition packets) split across both HWDGE
      queues; x over the gpsimd SWDGE queue. Row permutations induced by the
      contiguous layout are absorbed into the j-order (reduction invariant)
      and undone for x by one on-chip reorder copy at the end.
    - Per chunk: DVE square+reduce -> ||d_j||^2, ACT -> w_j = exp(-C nrm + SW)
      (bf16), PE transpose + DVE cast -> dataT in fp16.
    - Main loop, 2 sweeps over i-chunks of 512, groups of 3 j-tiles:
      3 fp16 matmuls [j=128, i=512] (1 cyc/col), one merged ACT exp over
      [128, 1536] psum with constant bias -SE -> E (bf16), then 3 matvecs
      with w_jt as the stationary operand accumulate sum_j w_j E_ij into a
      persistent PSUM bank. Matvecs are emitted one group late so the PE
      never waits on the current group's ACT.
"""
import math
from contextlib import ExitStack

import numpy as np

from concourse import bacc, mybir, tile
from concourse.bass_utils import run_bass_kernel_spmd
from concourse import masks

N, M, D = 8192, 8192, 128
NCORES = 8
NS = N // NCORES            # 1024 x-rows per core
P = 128                     # partitions
NT_X = NS // P              # 8 x tiles
NT_D = M // P               # 64 data tiles
NCHUNK = 16                 # data DMA chunks
TPC = NT_D // NCHUNK        # 4 tiles per chunk
GRP = 3                     # j-tiles per merged ACT group

C = 0.5 / math.sqrt(2.0 * math.pi)          # 0.19947114020071635
TWO_C = 2.0 * C                             # 0.3989422804014327
LNM = math.log(float(M))                    # ln 8192
SHIFT_E = 21.0
SHIFT_W = 25.5

F32 = mybir.dt.float32
F32R = mybir.dt.float32r
F16 = mybir.dt.float16
BF16 = mybir.dt.bfloat16
EXP = mybir.ActivationFunctionType.Exp

_CACHED_NC = None





def _build():
    nc = bacc.Bacc("TRN2", target_bir_lowering=False, debug=False)
    x_d = nc.dram_tensor("x", [NS, D], F32, kind="ExternalInput")
    d_d = nc.dram_tensor("data", [M, D], F32, kind="ExternalInput")
    o_d = nc.dram_tensor("out", [1, NS], F32, kind="ExternalOutput")

    # Contiguous loads: partition p takes a contiguous run of rows, one 4KB
    # DMA packet per partition.
    x_re = x_d.ap().rearrange("(p r) d -> p r d", p=P)         # [128, 8, 128]
    d_re = d_d.ap().rearrange("(ch p r) d -> ch p r d", ch=NCHUNK, p=P)

    with tile.TileContext(nc) as tc, ExitStack() as ctx:
        const_pool = ctx.enter_context(tc.tile_pool(name="const", bufs=1))
        dT_pool = ctx.enter_context(tc.tile_pool(name="dT", bufs=1))
        xbuf_pool = ctx.enter_context(tc.tile_pool(name="xbuf", bufs=1))
        drow_pool = ctx.enter_context(tc.tile_pool(name="drow", bufs=8))
        scr_pool = ctx.enter_context(tc.tile_pool(name="scr", bufs=2))
        e_pool = ctx.enter_context(tc.tile_pool(name="e", bufs=3))
        out_pool = ctx.enter_context(tc.tile_pool(name="outp", bufs=1))
        ps_main = ctx.enter_context(tc.tile_pool(name="psm", bufs=2, space="PSUM"))
        ps_acc = ctx.enter_context(tc.tile_pool(name="psa", bufs=1, space="PSUM"))
        ps_tr = ctx.enter_context(tc.tile_pool(name="pst", bufs=1, space="PSUM"))

        # Constants first so gpsimd finishes before its SWDGE queue starts.
        ident = const_pool.tile([P, P], F16, tag="ident")
        masks.make_identity(nc, ident[:])
        ones_f = const_pool.tile([P, 1], F32, tag="onesf")
        nc.gpsimd.memset(ones_f[:], 1.0)
        ones_r = const_pool.tile([P, 1], F32R, tag="ones")
        nc.vector.tensor_copy(ones_r[:], ones_f[:])
        be_tile = const_pool.tile([P, 1], F32, tag="be")
        nc.gpsimd.memset(be_tile[:], -SHIFT_E)
        bw_tile = const_pool.tile([P, 1], F32, tag="bw")
        nc.gpsimd.memset(bw_tile[:], SHIFT_W)
        bx_tile = const_pool.tile([1, 1], F32, tag="bx")
        nc.gpsimd.memset(bx_tile[:], -LNM - (SHIFT_W - SHIFT_E))

        dataT = dT_pool.tile([P, M], F16, tag="dataT")           # 16KB/part
        xT = xbuf_pool.tile([P, NS], F16, tag="xT")
        xsqT = xbuf_pool.tile([P, NS], F32R, tag="xsqT")
        xrow = xbuf_pool.tile([P, NT_X, P], F32, tag="xrow")
        dnsq = const_pool.tile([P, NT_D], F32, tag="dnsq")
        w16 = const_pool.tile([P, NT_D], BF16, tag="w16")
        exf = out_pool.tile([1, NS], F32, tag="exf")
        dens = out_pool.tile([1, NS], F32, tag="dens")

        # ---- DMA issue: x split over two queues, data over all three ----
        nc.gpsimd.dma_start(xrow[:, 0:4, :], x_re[:, 0:4, :])
        nc.sync.dma_start(xrow[:, 4:8, :], x_re[:, 4:8, :])
        drows = []
        qmap = [nc.gpsimd, nc.sync, nc.scalar]
        for ch in range(NCHUNK):
            drow = drow_pool.tile([P, TPC, P], F32, tag="drow")
            qmap[ch % 3].dma_start(drow[:], d_re[ch])
            drows.append(drow)

        # ---- x prologue: transposes to fp16 xT ----
        xrow16 = xbuf_pool.tile([P, NT_X, P], F16, tag="xrow16")
        nc.vector.tensor_copy(xrow16[:, 0:4, :], xrow[:, 0:4, :])
        nc.vector.tensor_copy(xrow16[:, 4:8, :], xrow[:, 4:8, :])
        for t in range(NT_X):
            tr = ps_tr.tile([P, P], F16, tag="tr")
            nc.tensor.transpose(tr[:], xrow16[:, t, :], ident[:])
            nc.vector.tensor_copy(xT[:, t * P:(t + 1) * P], tr[:])

        # ---- data prologue: stream chunks; norms, w, transposes ----
        for ch in range(NCHUNK):
            drow = drows[ch]
            csl = slice(ch * TPC, (ch + 1) * TPC)
            scr = scr_pool.tile([P, TPC, P], F32, tag="scr")
            nc.vector.tensor_mul(scr[:], drow[:], drow[:])
            nc.vector.tensor_reduce(
                dnsq[:, csl], scr[:],
                axis=mybir.AxisListType.X, op=mybir.AluOpType.add)
            nc.scalar.activation(w16[:, csl], dnsq[:, csl], EXP,
                                 bias=bw_tile[:], scale=-C)
            drow16 = scr_pool.tile([P, TPC, P], F16, tag="drow16")
            nc.vector.tensor_copy(drow16[:], drow[:])
            for k in range(TPC):
                s = ch * TPC + k
                tr = ps_tr.tile([P, P], F16, tag="tr")
                nc.tensor.transpose(tr[:], drow16[:, k, :], ident[:])
                nc.vector.tensor_copy(dataT[:, s * P:(s + 1) * P], tr[:])

        # x-norm factor (low priority: only the final multiply needs it)
        nc.vector.tensor_mul(xsqT[:], xT[:], xT[:])
        pmx = ps_main.tile([P, GRP * 512], F32, tag="pm")
        for c2 in range(2):
            sl = slice(c2 * 512, (c2 + 1) * 512)
            nc.tensor.matmul(pmx[0:1, sl], ones_r[:], xsqT[:, sl],
                             start=True, stop=True)
        nc.scalar.activation(exf[:], pmx[0:1, 0:NS], EXP,
                             bias=bx_tile[:], scale=-C)

        # ---- main: 2 sweeps over i-chunks, groups of GRP j-tiles ----
        groups = [list(range(g, min(g + GRP, NT_D)))
                  for g in range(0, NT_D, GRP)]
        for ic in range(2):
            isl = slice(ic * 512, (ic + 1) * 512)
            acc = ps_acc.tile([1, 512], F32, tag="acc")
            pend = None          # (e_tile, jts) awaiting matvecs
            for jts in groups:
                pm = ps_main.tile([P, GRP * 512], F32, tag="pm")
                for k, jt in enumerate(jts):
                    nc.tensor.matmul(pm[:, k * 512:(k + 1) * 512],
                                     dataT[:, jt * P:(jt + 1) * P],
                                     xT[:, isl], start=True, stop=True)
                e = e_pool.tile([P, GRP * 512], BF16, tag="e")
                width = len(jts) * 512
                nc.scalar.activation(e[:, 0:width], pm[:, 0:width], EXP,
                                     bias=be_tile[:], scale=TWO_C)
                if pend is not None:
                    pe_, pjts = pend
                    for k, jt in enumerate(pjts):
                        nc.tensor.matmul(
                            acc[:], w16[:, jt:jt + 1],
                            pe_[:, k * 512:(k + 1) * 512],
                            start=(jt == 0), stop=False,
                            skip_group_check=True)
                pend = (e, jts)
            pe_, pjts = pend
            for k, jt in enumerate(pjts):
                nc.tensor.matmul(acc[:], w16[:, jt:jt + 1],
                                 pe_[:, k * 512:(k + 1) * 512],
                                 start=False, stop=(jt == NT_D - 1),
                                 skip_group_check=True)
            # sweep epilogue: scale by x-norm factor; frees acc for sweep 2
            nc.vector.tensor_mul(dens[:, isl], acc[:], exf[:, isl])

        # ---- undo the x row permutation, write out ----
        dens_o = out_pool.tile([1, NS], F32, tag="dens_o")
        nc.vector.tensor_copy(
            dens_o[:], dens[:].rearrange("o (r p) -> o p r", p=P))
        nc.sync.dma_start(o_d.ap(), dens_o[:])

    nc.compile()
    return nc


def kernel(x, data):
    global _CACHED_NC
    x = np.ascontiguousarray(np.asarray(x, dtype=np.float32))
    data = np.ascontiguousarray(np.asarray(data, dtype=np.float32))
    assert x.shape == (N, D) and data.shape == (M, D)

    if _CACHED_NC is None:
        _CACHED_NC = _build()
    nc = _CACHED_NC

    in_maps = [
        {"x": x[c * NS:(c + 1) * NS], "data": data} for c in range(NCORES)
    ]
    res = run_bass_kernel_spmd(nc, in_maps, list(range(NCORES)))
    dens = np.concatenate(
        [np.asarray(res.results[c]["out"]).reshape(NS) for c in range(NCORES)]
    )
    return dens.reshape(N, 1).astype(np.float32)


if __name__ == "__main__":
    rng = np.random.default_rng(0)
    x = rng.standard_normal((N, D), dtype=np.float32)
    data = rng.standard_normal((M, D), dtype=np.float32)
    out = kernel(x, data)
    print("kernel out", out.shape, out[:4, 0])


# revision 32
# speedup vs baseline: 1.0736x; 1.0736x over previous
"""Trainium2 Bass kernel for differentiable KDE (Gaussian kernel density estimate).

Math (h = 1):
    sq[i,j]    = ||x_i||^2 + ||d_j||^2 - 2 x_i.d_j
    density[i] = mean_j exp(-C * sq[i,j]),   C = 0.5 / sqrt(2*pi)
               = exp(-C||x_i||^2 - lnM - (SW-SE))
                 * sum_j [exp(-C||d_j||^2 + SW) * exp(2C x_i.d_j - SE)]
    with constant range-shifts SE, SW so both exp factors fit bf16 comfortably.

Sharding: data-parallel over x rows (1024 per core), data replicated.

Per-core pipeline (j = data row as PSUM partition, i = x row as free dim):
    - Contiguous DMA chunks (4KB/partition packets) split across both HWDGE
      queues; x over the gpsimd SWDGE queue. Row permutations induced by the
      contiguous layout are absorbed into the j-order (reduction invariant)
      and undone for x by one on-chip reorder copy at the end.
    - Per chunk: DVE square+reduce -> ||d_j||^2, ACT -> w_j = exp(-C nrm + SW)
      (bf16), PE transpose + DVE cast -> dataT in fp16.
    - Main loop, 2 sweeps over i-chunks of 512, groups of 3 j-tiles:
      3 fp16 matmuls [j=128, i=512] (1 cyc/col), one merged ACT exp over
      [128, 1536] psum with constant bias -SE -> E (bf16), then 3 matvecs
      with w_jt as the stationary operand accumulate sum_j w_j E_ij into a
      persistent PSUM bank. Matvecs are emitted one group late so the PE
      never waits on the current group's ACT.
"""
import math
from contextlib import ExitStack

import numpy as np

from concourse import bacc, mybir, tile
from concourse.bass_utils import run_bass_kernel_spmd
from concourse import masks

N, M, D = 8192, 8192, 128
NCORES = 8
NS = N // NCORES            # 1024 x-rows per core
P = 128                     # partitions
NT_X = NS // P              # 8 x tiles
NT_D = M // P               # 64 data tiles
NCHUNK = 8                  # data DMA chunks
TPC = NT_D // NCHUNK        # 8 tiles per chunk
GRP = 3                     # j-tiles per merged ACT group

C = 0.5 / math.sqrt(2.0 * math.pi)          # 0.19947114020071635
TWO_C = 2.0 * C                             # 0.3989422804014327
LNM = math.log(float(M))                    # ln 8192
SHIFT_E = 21.0
SHIFT_W = 25.5

F32 = mybir.dt.float32
F32R = mybir.dt.float32r
F16 = mybir.dt.float16
BF16 = mybir.dt.bfloat16
EXP = mybir.ActivationFunctionType.Exp

_CACHED_NC = None


def _build():
    nc = bacc.Bacc("TRN2", target_bir_lowering=False, debug=False)
    x_d = nc.dram_tensor("x", [NS, D], F32, kind="ExternalInput")
    d_d = nc.dram_tensor("data", [M, D], F32, kind="ExternalInput")
    o_d = nc.dram_tensor("out", [1, NS], F32, kind="ExternalOutput")

    # Contiguous loads: partition p takes a contiguous run of rows, one 4KB
    # DMA packet per partition.
    x_re = x_d.ap().rearrange("(p r) d -> p r d", p=P)         # [128, 8, 128]
    d_re = d_d.ap().rearrange("(ch p r) d -> ch p r d", ch=NCHUNK, p=P)

    with tile.TileContext(nc) as tc, ExitStack() as ctx:
        const_pool = ctx.enter_context(tc.tile_pool(name="const", bufs=1))
        dT_pool = ctx.enter_context(tc.tile_pool(name="dT", bufs=1))
        xbuf_pool = ctx.enter_context(tc.tile_pool(name="xbuf", bufs=1))
        drow_pool = ctx.enter_context(tc.tile_pool(name="drow", bufs=8))
        scr_pool = ctx.enter_context(tc.tile_pool(name="scr", bufs=2))
        e_pool = ctx.enter_context(tc.tile_pool(name="e", bufs=3))
        out_pool = ctx.enter_context(tc.tile_pool(name="outp", bufs=1))
        ps_main = ctx.enter_context(tc.tile_pool(name="psm", bufs=2, space="PSUM"))
        ps_acc = ctx.enter_context(tc.tile_pool(name="psa", bufs=1, space="PSUM"))
        ps_tr = ctx.enter_context(tc.tile_pool(name="pst", bufs=1, space="PSUM"))

        # Constants first so gpsimd finishes before its SWDGE queue starts.
        ident = const_pool.tile([P, P], F16, tag="ident")
        masks.make_identity(nc, ident[:])
        ones_f = const_pool.tile([P, 1], F32, tag="onesf")
        nc.gpsimd.memset(ones_f[:], 1.0)
        ones_r = const_pool.tile([P, 1], F32R, tag="ones")
        nc.vector.tensor_copy(ones_r[:], ones_f[:])
        be_tile = const_pool.tile([P, 1], F32, tag="be")
        nc.gpsimd.memset(be_tile[:], -SHIFT_E)
        bw_tile = const_pool.tile([P, 1], F32, tag="bw")
        nc.gpsimd.memset(bw_tile[:], SHIFT_W)
        bx_tile = const_pool.tile([1, 1], F32, tag="bx")
        nc.gpsimd.memset(bx_tile[:], -LNM - (SHIFT_W - SHIFT_E))

        dataT = dT_pool.tile([P, M], F16, tag="dataT")           # 16KB/part
        xT = xbuf_pool.tile([P, NS], F16, tag="xT")
        xsqT = xbuf_pool.tile([P, NS], F32R, tag="xsqT")
        xrow = xbuf_pool.tile([P, NT_X, P], F32, tag="xrow")
        dnsq = const_pool.tile([P, NT_D], F32, tag="dnsq")
        w16 = const_pool.tile([P, NT_D], BF16, tag="w16")
        exf = out_pool.tile([1, NS], F32, tag="exf")
        dens = out_pool.tile([1, NS], F32, tag="dens")

        # ---- DMA issue: x on SWDGE, data chunks over all three queues ----
        nc.gpsimd.dma_start(xrow[:], x_re)
        drows = []
        # sync's HWDGE queue is measurably slow; give it only the last-
        # consumed chunk. gpsimd SWDGE is fastest but shares with the x load.
        qmap = [nc.scalar, nc.gpsimd, nc.scalar, nc.gpsimd,
                nc.scalar, nc.gpsimd, nc.scalar, nc.sync]
        for ch in range(NCHUNK):
            drow = drow_pool.tile([P, TPC, P], F32, tag="drow")
            qmap[ch].dma_start(drow[:], d_re[ch])
            drows.append(drow)

        # ---- x prologue: transpose, squared-norm factor in [1, NS] ----
        xrow16 = xbuf_pool.tile([P, NT_X, P], F16, tag="xrow16")
        nc.vector.tensor_copy(xrow16[:], xrow[:])
        for t in range(NT_X):
            tr = ps_tr.tile([P, P], F16, tag="tr")
            nc.tensor.transpose(tr[:], xrow16[:, t, :], ident[:])
            nc.vector.tensor_copy(xT[:, t * P:(t + 1) * P], tr[:])
        nc.vector.tensor_mul(xsqT[:], xT[:], xT[:])
        pmx = ps_main.tile([P, GRP * 512], F32, tag="pm")
        for c2 in range(2):
            sl = slice(c2 * 512, (c2 + 1) * 512)
            nc.tensor.matmul(pmx[0:1, sl], ones_r[:], xsqT[:, sl],
                             start=True, stop=True)
        nc.scalar.activation(exf[:], pmx[0:1, 0:NS], EXP,
                             bias=bx_tile[:], scale=-C)

        # ---- data prologue: stream chunks; norms, w, transposes ----
        for ch in range(NCHUNK):
            drow = drows[ch]
            csl = slice(ch * TPC, (ch + 1) * TPC)
            scr = scr_pool.tile([P, TPC, P], F32, tag="scr")
            nc.vector.tensor_mul(scr[:], drow[:], drow[:])
            nc.vector.tensor_reduce(
                dnsq[:, csl], scr[:],
                axis=mybir.AxisListType.X, op=mybir.AluOpType.add)
            nc.scalar.activation(w16[:, csl], dnsq[:, csl], EXP,
                                 bias=bw_tile[:], scale=-C)
            drow16 = scr_pool.tile([P, TPC, P], F16, tag="drow16")
            nc.vector.tensor_copy(drow16[:], drow[:])
            for k in range(TPC):
                s = ch * TPC + k
                tr = ps_tr.tile([P, P], F16, tag="tr")
                nc.tensor.transpose(tr[:], drow16[:, k, :], ident[:])
                nc.vector.tensor_copy(dataT[:, s * P:(s + 1) * P], tr[:])

        # ---- main: 2 sweeps over i-chunks, groups of GRP j-tiles ----
        groups = [list(range(g, min(g + GRP, NT_D)))
                  for g in range(0, NT_D, GRP)]
        for ic in range(2):
            isl = slice(ic * 512, (ic + 1) * 512)
            acc = ps_acc.tile([1, 512], F32, tag="acc")
            pend = None          # (e_tile, jts) awaiting matvecs
            for jts in groups:
                pm = ps_main.tile([P, GRP * 512], F32, tag="pm")
                for k, jt in enumerate(jts):
                    nc.tensor.matmul(pm[:, k * 512:(k + 1) * 512],
                                     dataT[:, jt * P:(jt + 1) * P],
                                     xT[:, isl], start=True, stop=True)
                e = e_pool.tile([P, GRP * 512], BF16, tag="e")
                width = len(jts) * 512
                nc.scalar.activation(e[:, 0:width], pm[:, 0:width], EXP,
                                     bias=be_tile[:], scale=TWO_C)
                if pend is not None:
                    pe_, pjts = pend
                    for k, jt in enumerate(pjts):
                        nc.tensor.matmul(
                            acc[:], w16[:, jt:jt + 1],
                            pe_[:, k * 512:(k + 1) * 512],
                            start=(jt == 0), stop=False,
                            skip_group_check=True)
                pend = (e, jts)
            pe_, pjts = pend
            for k, jt in enumerate(pjts):
                nc.tensor.matmul(acc[:], w16[:, jt:jt + 1],
                                 pe_[:, k * 512:(k + 1) * 512],
                                 start=False, stop=(jt == NT_D - 1),
                                 skip_group_check=True)
            # sweep epilogue: scale by x-norm factor; frees acc for sweep 2
            nc.vector.tensor_mul(dens[:, isl], acc[:], exf[:, isl])

        # ---- undo the x row permutation, write out ----
        dens_o = out_pool.tile([1, NS], F32, tag="dens_o")
        nc.vector.tensor_copy(
            dens_o[:], dens[:].rearrange("o (r p) -> o p r", p=P))
        nc.sync.dma_start(o_d.ap(), dens_o[:])

    nc.compile()
    return nc


def kernel(x, data):
    global _CACHED_NC
    x = np.ascontiguousarray(np.asarray(x, dtype=np.float32))
    data = np.ascontiguousarray(np.asarray(data, dtype=np.float32))
    assert x.shape == (N, D) and data.shape == (M, D)

    if _CACHED_NC is None:
        _CACHED_NC = _build()
    nc = _CACHED_NC

    in_maps = [
        {"x": x[c * NS:(c + 1) * NS], "data": data} for c in range(NCORES)
    ]
    res = run_bass_kernel_spmd(nc, in_maps, list(range(NCORES)))
    dens = np.concatenate(
        [np.asarray(res.results[c]["out"]).reshape(NS) for c in range(NCORES)]
    )
    return dens.reshape(N, 1).astype(np.float32)


if __name__ == "__main__":
    rng = np.random.default_rng(0)
    x = rng.standard_normal((N, D), dtype=np.float32)
    data = rng.standard_normal((M, D), dtype=np.float32)
    out = kernel(x, data)
    print("kernel out", out.shape, out[:4, 0])
